# revision 14
# baseline (speedup 1.0000x reference)
"""Trainium2 Bass kernel for DepthwiseSeparableConv3d (inference).

Problem: x[2,48,48,48,64] -> dw3x3x3 depthwise + BN + ReLU -> 1x1x1 conv
(64->128) + BN + ReLU -> z[2,48,48,48,128], all f32.

Strategy (8 NeuronCores, data-parallel over (b,d) slabs, 12 slabs/core):
 - Host pre-pads D (1-slab halo per side, zero at batch edges) and H/W
   (SAME padding) so the device kernel is a pure VALID 3x3x3 conv.
 - Depthwise conv runs on TensorE as a block-Toeplitz matmul:
   K=112 partitions = (8 channels x 14 W-inputs),
   M=96 partitions  = (8 channels x 12 W-outputs).
   The 3 W-taps live in the Toeplitz weight; the 9 (dz,dy) taps are
   PSUM-accumulated matmuls against free-dim-shifted views of the same
   SBUF tile (shifting free dims is free in an access pattern).
 - BN1+ReLU is one ScalarE activation (per-partition scale/bias).
 - A per-channel-group SBUF->SBUF DMA regroups (c,w)-partitions into
   pure-channel partitions (contiguous 576-elem blocks on both sides).
 - Pointwise 64->128 is a plain matmul; BN2+ReLU is one activation.
 - Output stays [f, positions] on device; host transposes to NDHWC.
"""

import os
import sys

for _p in ("/opt/trn_rl_repo", "/opt/pypackages"):
    if _p not in sys.path:
        sys.path.insert(0, _p)

import numpy as np
import ml_dtypes

import concourse.bass as bass
import concourse.tile as tile
from concourse import bacc, mybir
from concourse.bass_utils import run_bass_kernel_spmd

# ----- problem constants (hardcoded per spec) -----
B, D, H, W, C, F = 2, 48, 48, 48, 64, 128
EPS = 1e-3
N_CORES = 8
DPC = (B * D) // N_CORES          # d-slabs per core = 12
CG = 8                            # channels per depthwise group
NG = C // CG                      # 8 groups
WT = 4                            # W tiles
WO = W // WT                      # 12 outputs per W tile
WI = WO + 2                       # 14 inputs per W tile
KP = CG * WI                      # 112 K partitions
MP = CG * WO                      # 96 M partitions
DH = DPC * H                      # 576 (d,h) positions per W value
NHALF = 2                         # split (d,h) into two 288-col matmuls
NCOL = DH // NHALF                # 288
NPOS = DPC * H * W                # 27648 positions per core
ZCHUNK = 4                        # PW chunks per output DMA

BF16 = mybir.dt.bfloat16
F32 = mybir.dt.float32

_COMPILED = None


def _build_bass():
    nc = bacc.Bacc("TRN2", target_bir_lowering=False, debug=False,
                   num_devices=N_CORES)

    xt_d = nc.dram_tensor("xt", [NG, KP, WT, DPC + 2, H + 2], BF16,
                          kind="ExternalInput").ap()
    # M padded 96->128 so Fast Weight Load kicks in (needs 128 cols)
    wt_d = nc.dram_tensor("wt", [NG, KP, 9, 128], BF16,
                          kind="ExternalInput").ap()
    # two stacked copies of pw (rows 0-63 / 64-127) for dual-stream PW
    pw_d = nc.dram_tensor("pwk", [2 * C, F], BF16, kind="ExternalInput").ap()
    s1_d = nc.dram_tensor("s1", [MP, NG], F32, kind="ExternalInput").ap()
    b1_d = nc.dram_tensor("b1", [MP, NG], F32, kind="ExternalInput").ap()
    s2_d = nc.dram_tensor("s2", [F, 1], F32, kind="ExternalInput").ap()
    b2_d = nc.dram_tensor("b2", [F, 1], F32, kind="ExternalInput").ap()
    z_d = nc.dram_tensor("z", [F, NPOS], F32, kind="ExternalOutput").ap()

    with tile.TileContext(nc) as tc:
        with (
            tc.tile_pool(name="consts", bufs=1) as consts,
            tc.tile_pool(name="xt", bufs=NG) as xt_pool,
            tc.tile_pool(name="wt", bufs=NG) as wt_pool,
            tc.tile_pool(name="ybuf", bufs=3) as y_pool,
            tc.tile_pool(name="Ybig", bufs=1) as Y_pool,
            tc.tile_pool(name="zbuf", bufs=3) as z_pool,
        ):
            pw_sb = consts.tile([2 * C, F], BF16)
            nc.sync.dma_start(pw_sb[:], pw_d[:])
            s1_sb = consts.tile([MP, NG], F32)
            nc.sync.dma_start(s1_sb[:], s1_d[:])
            b1_sb = consts.tile([MP, NG], F32)
            nc.sync.dma_start(b1_sb[:], b1_d[:])
            s2_sb = consts.tile([F, 1], F32)
            nc.sync.dma_start(s2_sb[:], s2_d[:])
            b2_sb = consts.tile([F, 1], F32)
            nc.sync.dma_start(b2_sb[:], b2_d[:])

            # Y: depthwise output in pure-channel layout, duplicated on
            # partitions 64-127 for the second PW stream.
            # free order (w_o, t, d, h): w_global = t*WO + w_o
            Y = Y_pool.tile([2 * C, WO, WT, DPC, H], BF16)

            xg = []
            wg = []
            for g in range(NG):
                xg_t = xt_pool.tile([KP, WT, DPC + 2, H + 2], BF16, tag="xg")
                nc.sync.dma_start(xg_t[:], xt_d[g])
                wg_t = wt_pool.tile([KP, 9, 128], BF16, tag="wg")
                nc.sync.dma_start(wg_t[:], wt_d[g])
                xg.append(xg_t)
                wg.append(wg_t)

            with tc.tile_pool(name="psdw", bufs=4, space="PSUM") as ps_pool:
                for g in range(NG):
                    yg = y_pool.tile([MP, WT, DPC, H], BF16, tag="yg")
                    # one tile per t; nh halves bank-aligned at 512
                    ps = [ps_pool.tile([128, NHALF, 512], F32, tag="ps",
                                       name=f"ps_{t}") for t in range(WT)]
                    for izy, (dz, dy) in enumerate(
                            (a, b) for a in range(3) for b in range(3)):
                        for t in range(WT):
                            for nh in range(NHALF):
                                d0 = nh * (DPC // NHALF)
                                rhs = xg[g][:, t,
                                            dz + d0: dz + d0 + DPC // NHALF,
                                            dy: dy + H]
                                nc.tensor.matmul(
                                    ps[t][:, nh, 0:NCOL],
                                    wg[g][:, izy, :],
                                    rhs,
                                    start=(izy == 0),
                                    stop=(izy == 8),
                                )
                    for t in range(WT):
                        out_v = yg[:, t].rearrange("c d h -> c (d h)") \
                                        .rearrange("c (n r) -> c n r",
                                                   n=NHALF, r=NCOL)
                        nc.scalar.activation(
                            out_v,
                            ps[t][0:MP, :, 0:NCOL],
                            mybir.ActivationFunctionType.Relu,
                            bias=b1_sb[:, g: g + 1],
                            scale=s1_sb[:, g: g + 1],
                        )
                    # regroup (c,w)-partitions -> channel partitions,
                    # duplicated into both PW stream halves.
                    nc.sync.dma_start(Y[g * CG:(g + 1) * CG], yg[:])
                    nc.sync.dma_start(Y[C + g * CG: C + (g + 1) * CG], yg[:])

            # pointwise + BN2 + ReLU, dual-stream, chunked over positions
            n_chunks = NPOS // NCOL          # 96
            half = n_chunks // 2             # 48
            Yf = Y.rearrange("c w t d h -> c (w t d h)")
            zf = z_d.rearrange("f (j n) -> f j n", j=n_chunks, n=NCOL)
            with tc.tile_pool(name="pspw", bufs=2, space="PSUM") as pw_pool:
                for q in range(half // ZCHUNK):      # 12 quads per stream
                    for s in range(2):               # stream A / B
                        base = s * half + q * ZCHUNK
                        pps = pw_pool.tile([F, ZCHUNK, 512], F32, tag="pwps",
                                           name=f"pps_{s}")
                        zt = z_pool.tile([F, ZCHUNK, NCOL], F32, tag="zt")
                        for jj in range(ZCHUNK):
                            j = base + jj
                            nc.tensor.matmul(
                                pps[:, jj, 0:NCOL],
                                pw_sb[s * C:(s + 1) * C],
                                Yf[s * C:(s + 1) * C,
                                   j * NCOL:(j + 1) * NCOL],
                                start=True, stop=True)
                        nc.scalar.activation(
                            zt[:], pps[:, :, 0:NCOL],
                            mybir.ActivationFunctionType.Relu,
                            bias=b2_sb[:, 0:1], scale=s2_sb[:, 0:1])
                        nc.sync.dma_start(zf[:, base: base + ZCHUNK, :],
                                          zt[:])

    nc.compile()
    return nc


def _prep_inputs(x, dw_kernel, dw_bias, bn1_gamma, bn1_beta, bn1_mean,
                 bn1_var, pw_kernel, pw_bias, bn2_gamma, bn2_beta, bn2_mean,
                 bn2_var):
    """Build per-core input maps (numpy only, off the device clock)."""
    x = np.asarray(x, np.float32)
    dw_kernel = np.asarray(dw_kernel, np.float32)
    dw_bias = np.asarray(dw_bias, np.float32)
    bn1_gamma = np.asarray(bn1_gamma, np.float32)
    bn1_beta = np.asarray(bn1_beta, np.float32)
    bn1_mean = np.asarray(bn1_mean, np.float32)
    bn1_var = np.asarray(bn1_var, np.float32)
    pw_kernel = np.asarray(pw_kernel, np.float32)
    pw_bias = np.asarray(pw_bias, np.float32)
    bn2_gamma = np.asarray(bn2_gamma, np.float32)
    bn2_beta = np.asarray(bn2_beta, np.float32)
    bn2_mean = np.asarray(bn2_mean, np.float32)
    bn2_var = np.asarray(bn2_var, np.float32)
    a1 = bn1_gamma / np.sqrt(bn1_var + EPS)
    c1 = a1 * (dw_bias - bn1_mean) + bn1_beta
    a2 = bn2_gamma / np.sqrt(bn2_var + EPS)
    c2 = a2 * (pw_bias - bn2_mean) + bn2_beta

    # depthwise Toeplitz weights: [NG, KP, 9, 128] (M padded 96->128)
    dw = dw_kernel[:, :, :, 0, :]                       # [3,3,3,C]
    wt = np.zeros((NG, KP, 9, 128), np.float32)
    for ci in range(CG):
        for wo in range(WO):
            for dx in range(3):
                # wt[g, ci*WI + wo+dx, (dz*3+dy), ci*WO + wo] = dw[dz,dy,dx,c]
                wt[:, ci * WI + wo + dx, :, ci * WO + wo] = (
                    dw[:, :, dx, :].reshape(9, NG, CG)[:, :, ci].T)
    wt = wt.astype(ml_dtypes.bfloat16)

    # scale/bias vectors in (c-major, w) partition order: m = ci*WO + wo
    s1 = np.zeros((MP, NG), np.float32)
    b1 = np.zeros((MP, NG), np.float32)
    for g in range(NG):
        for ci in range(CG):
            s1[ci * WO:(ci + 1) * WO, g] = a1[g * CG + ci]
            b1[ci * WO:(ci + 1) * WO, g] = c1[g * CG + ci]

    pwk = np.concatenate([pw_kernel, pw_kernel], axis=0) \
            .astype(ml_dtypes.bfloat16)
    s2 = a2.reshape(F, 1).astype(np.float32)
    b2 = c2.reshape(F, 1).astype(np.float32)

    # x padded once globally: [B, D+2, H+2, W+2, C]
    xp = np.zeros((B, D + 2, H + 2, W + 2, C), np.float32)
    xp[:, 1:-1, 1:-1, 1:-1, :] = x
    xp = xp.astype(ml_dtypes.bfloat16)

    in_maps = []
    for core in range(N_CORES):
        b = (core * DPC) // D
        d0 = (core * DPC) % D
        sl = xp[b, d0: d0 + DPC + 2]                    # [14, 50, 50, C]
        # xt[g, ci*WI+wi, t, d, h] = sl[d, h, 12t+wi, 8g+ci]
        xt = np.ascontiguousarray(sl.transpose(3, 2, 0, 1))  # [C, w50, d, h]
        # build overlapping w-tiles: index w = t*WO + wi
        idx = (np.arange(WT)[:, None] * WO + np.arange(WI)[None, :]).ravel()
        xt = xt[:, idx]                                 # [C, WT*WI, d, h]
        xt = xt.reshape(NG, CG, WT, WI, DPC + 2, H + 2) \
               .transpose(0, 1, 3, 2, 4, 5) \
               .reshape(NG, KP, WT, DPC + 2, H + 2)
        in_maps.append({
            "xt": np.ascontiguousarray(xt),
            "wt": wt, "pwk": pwk, "s1": s1, "b1": b1, "s2": s2, "b2": b2,
        })
    return in_maps


def _gather_output(results):
    z = np.empty((B, D, H, W, F), np.float32)
    for core in range(N_CORES):
        b = (core * DPC) // D
        d0 = (core * DPC) % D
        zc = results[core]["z"]                         # [F, NPOS]
        # free order was (w_o, t, d, h); w_global = t*WO + w_o
        zc = zc.reshape(F, WO, WT, DPC, H).transpose(3, 4, 2, 1, 0)
        z[b, d0: d0 + DPC] = zc.reshape(DPC, H, W, F)
    return z


def kernel(**inputs):
    global _COMPILED
    if _COMPILED is None:
        _COMPILED = _build_bass()
    in_maps = _prep_inputs(**inputs)
    res = run_bass_kernel_spmd(_COMPILED, in_maps,
                               core_ids=list(range(N_CORES)))
    return _gather_output(res.results)


if __name__ == "__main__":
    pass


# revision 24
# speedup vs baseline: 1.0092x; 1.0092x over previous
"""Trainium2 Bass kernel for DepthwiseSeparableConv3d (inference).

Problem: x[2,48,48,48,64] -> dw3x3x3 depthwise + BN + ReLU -> 1x1x1 conv
(64->128) + BN + ReLU -> z[2,48,48,48,128], all f32.

Strategy (8 NeuronCores, data-parallel over (b,d) slabs, 12 slabs/core):
 - Host pre-pads D (1-slab halo per side, zero at batch edges) and H/W
   (SAME padding) so the device kernel is a pure VALID 3x3x3 conv.
 - Depthwise conv runs on TensorE as a block-Toeplitz matmul:
   K=112 partitions = (8 channels x 14 W-inputs),
   M=96 partitions  = (8 channels x 12 W-outputs).
   The 3 W-taps live in the Toeplitz weight; the 9 (dz,dy) taps are
   PSUM-accumulated matmuls against free-dim-shifted views of the same
   SBUF tile (shifting free dims is free in an access pattern).
 - BN1+ReLU is one ScalarE activation (per-partition scale/bias).
 - A per-channel-group SBUF->SBUF DMA regroups (c,w)-partitions into
   pure-channel partitions (contiguous 576-elem blocks on both sides).
 - Pointwise 64->128 is a plain matmul; BN2+ReLU is one activation.
 - Output stays [f, positions] on device; host transposes to NDHWC.
"""

import os
import sys

for _p in ("/opt/trn_rl_repo", "/opt/pypackages"):
    if _p not in sys.path:
        sys.path.insert(0, _p)

import numpy as np
import ml_dtypes

import concourse.bass as bass
import concourse.tile as tile
from concourse import bacc, mybir
from concourse.bass_utils import run_bass_kernel_spmd

# ----- problem constants (hardcoded per spec) -----
B, D, H, W, C, F = 2, 48, 48, 48, 64, 128
EPS = 1e-3
N_CORES = 8
DPC = (B * D) // N_CORES          # d-slabs per core = 12
CG = 8                            # channels per depthwise group
NG = C // CG                      # 8 groups
CQ = 4                            # channels per stream (half group)
WT = 4                            # W tiles
WO = W // WT                      # 12 outputs per W tile
WI = WO + 2                       # 14 inputs per W tile
KP = CQ * WI                      # 56 K partitions per stream
MP = CQ * WO                      # 48 M partitions per stream
DH = DPC * H                      # 576 (d,h) positions per W value
NHALF = 2                         # split (d,h) into two 288-col matmuls
NCOL = DH // NHALF                # 288
NPOS = DPC * H * W                # 27648 positions per core
ZCHUNK = 4                        # PW chunks per output DMA

BF16 = mybir.dt.bfloat16
F32 = mybir.dt.float32

_COMPILED = None


def _build_bass():
    nc = bacc.Bacc("TRN2", target_bir_lowering=False, debug=False,
                   num_devices=N_CORES)

    # dual-stream layout: stream A on partitions 0-55 (out 0-47),
    # stream B on partitions 64-119 (out 64-111); pad rows zero.
    xt_d = nc.dram_tensor("xt", [NG, 128, WT, DPC + 2, H + 2], BF16,
                          kind="ExternalInput").ap()
    wt_d = nc.dram_tensor("wt", [NG, 128, 9, MP], BF16,
                          kind="ExternalInput").ap()
    # two stacked copies of pw (rows 0-63 / 64-127) for dual-stream PW
    pw_d = nc.dram_tensor("pwk", [2 * C, F], BF16, kind="ExternalInput").ap()
    s1_d = nc.dram_tensor("s1", [128, NG], F32, kind="ExternalInput").ap()
    b1_d = nc.dram_tensor("b1", [128, NG], F32, kind="ExternalInput").ap()
    s2_d = nc.dram_tensor("s2", [F, 1], F32, kind="ExternalInput").ap()
    b2_d = nc.dram_tensor("b2", [F, 1], F32, kind="ExternalInput").ap()
    z_d = nc.dram_tensor("z", [F, NPOS], F32, kind="ExternalOutput").ap()

    with tile.TileContext(nc) as tc:
        with (
            tc.tile_pool(name="consts", bufs=1) as consts,
            tc.tile_pool(name="xt", bufs=NG) as xt_pool,
            tc.tile_pool(name="wt", bufs=NG) as wt_pool,
            tc.tile_pool(name="ybuf", bufs=3) as y_pool,
            tc.tile_pool(name="Ybig", bufs=1) as Y_pool,
            tc.tile_pool(name="zbuf", bufs=3) as z_pool,
        ):
            pw_sb = consts.tile([2 * C, F], BF16)
            nc.sync.dma_start(pw_sb[:], pw_d[:])
            s1_sb = consts.tile([128, NG], F32)
            nc.sync.dma_start(s1_sb[:], s1_d[:])
            b1_sb = consts.tile([128, NG], F32)
            nc.sync.dma_start(b1_sb[:], b1_d[:])
            s2_sb = consts.tile([F, 1], F32)
            nc.sync.dma_start(s2_sb[:], s2_d[:])
            b2_sb = consts.tile([F, 1], F32)
            nc.sync.dma_start(b2_sb[:], b2_d[:])

            # Y: depthwise output in pure-channel layout, duplicated on
            # partitions 64-127 for the second PW stream.
            # free order (w_o, t, d, h): w_global = t*WO + w_o
            Y = Y_pool.tile([2 * C, WO, WT, DPC, H], BF16)

            xg = []
            wg = []
            for g in range(NG):
                xg_t = xt_pool.tile([128, WT, DPC + 2, H + 2], BF16, tag="xg")
                nc.sync.dma_start(xg_t[:], xt_d[g])
                wg_t = wt_pool.tile([128, 9, MP], BF16, tag="wg")
                nc.sync.dma_start(wg_t[:], wt_d[g])
                xg.append(xg_t)
                wg.append(wg_t)

            with tc.tile_pool(name="psdw", bufs=2, space="PSUM") as ps_pool:
                for g in range(NG):
                    yg = y_pool.tile([128, WT, DPC, H], BF16, tag="yg")
                    for tp in range(2):              # t-pair {2tp, 2tp+1}
                        # 4 (t,nh) slices, bank-aligned at 512
                        pst = ps_pool.tile([128, 4, 512], F32, tag="ps",
                                           name=f"ps_{tp}")
                        for izy, (dz, dy) in enumerate(
                                (a, b) for a in range(3) for b in range(3)):
                            for s in range(2):       # stream A / B
                                p0 = s * 64
                                for tl in range(2):
                                    t = 2 * tp + tl
                                    for nh in range(NHALF):
                                        d0 = nh * (DPC // NHALF)
                                        rhs = xg[g][p0: p0 + KP, t,
                                                    dz + d0:
                                                    dz + d0 + DPC // NHALF,
                                                    dy: dy + H]
                                        nc.tensor.matmul(
                                            pst[p0: p0 + MP,
                                                tl * 2 + nh, 0:NCOL],
                                            wg[g][p0: p0 + KP, izy, :],
                                            rhs,
                                            start=(izy == 0),
                                            stop=(izy == 8),
                                            skip_group_check=True,
                                        )
                        for s in range(2):
                            p0 = s * 64
                            out_v = yg[p0: p0 + MP, 2 * tp: 2 * tp + 2] \
                                .rearrange("c t d h -> c t (d h)") \
                                .rearrange("c t (n r) -> c (t n) r",
                                           n=NHALF, r=NCOL)
                            nc.scalar.activation(
                                out_v,
                                pst[p0: p0 + MP, :, 0:NCOL],
                                mybir.ActivationFunctionType.Relu,
                                bias=b1_sb[p0: p0 + MP, g: g + 1],
                                scale=s1_sb[p0: p0 + MP, g: g + 1],
                            )
                    # regroup (c,w)-partitions -> channel partitions,
                    # duplicated into both PW stream halves.
                    for s in range(2):
                        src = yg[s * 64: s * 64 + MP]
                        c0 = g * CG + s * CQ
                        nc.sync.dma_start(Y[c0: c0 + CQ], src)
                        nc.sync.dma_start(Y[C + c0: C + c0 + CQ], src)

            # pointwise + BN2 + ReLU, dual-stream, chunked over positions
            n_chunks = NPOS // NCOL          # 96
            half = n_chunks // 2             # 48
            Yf = Y.rearrange("c w t d h -> c (w t d h)")
            zf = z_d.rearrange("f (j n) -> f j n", j=n_chunks, n=NCOL)
            with tc.tile_pool(name="pspw", bufs=2, space="PSUM") as pw_pool:
                for q in range(half // ZCHUNK):      # 12 quads per stream
                    for s in range(2):               # stream A / B
                        base = s * half + q * ZCHUNK
                        pps = pw_pool.tile([F, ZCHUNK, 512], F32, tag="pwps",
                                           name=f"pps_{s}")
                        zt = z_pool.tile([F, ZCHUNK, NCOL], F32, tag="zt")
                        for jj in range(ZCHUNK):
                            j = base + jj
                            nc.tensor.matmul(
                                pps[:, jj, 0:NCOL],
                                pw_sb[s * C:(s + 1) * C],
                                Yf[s * C:(s + 1) * C,
                                   j * NCOL:(j + 1) * NCOL],
                                start=True, stop=True)
                        nc.scalar.activation(
                            zt[:], pps[:, :, 0:NCOL],
                            mybir.ActivationFunctionType.Relu,
                            bias=b2_sb[:, 0:1], scale=s2_sb[:, 0:1])
                        nc.sync.dma_start(zf[:, base: base + ZCHUNK, :],
                                          zt[:])

    nc.compile()
    return nc


def _prep_inputs(x, dw_kernel, dw_bias, bn1_gamma, bn1_beta, bn1_mean,
                 bn1_var, pw_kernel, pw_bias, bn2_gamma, bn2_beta, bn2_mean,
                 bn2_var):
    """Build per-core input maps (numpy only, off the device clock)."""
    x = np.asarray(x, np.float32)
    dw_kernel = np.asarray(dw_kernel, np.float32)
    dw_bias = np.asarray(dw_bias, np.float32)
    bn1_gamma = np.asarray(bn1_gamma, np.float32)
    bn1_beta = np.asarray(bn1_beta, np.float32)
    bn1_mean = np.asarray(bn1_mean, np.float32)
    bn1_var = np.asarray(bn1_var, np.float32)
    pw_kernel = np.asarray(pw_kernel, np.float32)
    pw_bias = np.asarray(pw_bias, np.float32)
    bn2_gamma = np.asarray(bn2_gamma, np.float32)
    bn2_beta = np.asarray(bn2_beta, np.float32)
    bn2_mean = np.asarray(bn2_mean, np.float32)
    bn2_var = np.asarray(bn2_var, np.float32)
    a1 = bn1_gamma / np.sqrt(bn1_var + EPS)
    c1 = a1 * (dw_bias - bn1_mean) + bn1_beta
    a2 = bn2_gamma / np.sqrt(bn2_var + EPS)
    c2 = a2 * (pw_bias - bn2_mean) + bn2_beta

    # depthwise Toeplitz weights: [NG, 128, 9, MP]; stream A rows 0-55,
    # stream B rows 64-119; within a stream K = (ci*WI + wi), M = ci*WO+wo
    dw = dw_kernel[:, :, :, 0, :]                       # [3,3,3,C]
    wt = np.zeros((NG, 128, 9, MP), np.float32)
    for s in range(2):
        for ci in range(CQ):
            for wo in range(WO):
                for dx in range(3):
                    # [izy, g] for channel 8g + 4s + ci
                    v = dw[:, :, dx, :].reshape(9, NG, 2, CQ)[:, :, s, ci]
                    wt[:, s * 64 + ci * WI + wo + dx, :, ci * WO + wo] = v.T
    wt = wt.astype(ml_dtypes.bfloat16)

    # BN1 scale/bias per partition: rows s*64 + ci*WO + wo -> chan 8g+4s+ci
    s1 = np.zeros((128, NG), np.float32)
    b1 = np.zeros((128, NG), np.float32)
    for g in range(NG):
        for s in range(2):
            for ci in range(CQ):
                r0 = s * 64 + ci * WO
                s1[r0: r0 + WO, g] = a1[g * CG + s * CQ + ci]
                b1[r0: r0 + WO, g] = c1[g * CG + s * CQ + ci]

    pwk = np.concatenate([pw_kernel, pw_kernel], axis=0) \
            .astype(ml_dtypes.bfloat16)
    s2 = a2.reshape(F, 1).astype(np.float32)
    b2 = c2.reshape(F, 1).astype(np.float32)

    # x padded once globally: [B, D+2, H+2, W+2, C]
    xp = np.zeros((B, D + 2, H + 2, W + 2, C), np.float32)
    xp[:, 1:-1, 1:-1, 1:-1, :] = x
    xp = xp.astype(ml_dtypes.bfloat16)

    in_maps = []
    for core in range(N_CORES):
        b = (core * DPC) // D
        d0 = (core * DPC) % D
        sl = xp[b, d0: d0 + DPC + 2]                    # [14, 50, 50, C]
        # xt[g, s*64 + ci*WI+wi, t, d, h] = sl[d, h, 12t+wi, 8g+4s+ci]
        xv = np.ascontiguousarray(sl.transpose(3, 2, 0, 1))  # [C, w50, d, h]
        # build overlapping w-tiles: index w = t*WO + wi
        idx = (np.arange(WT)[:, None] * WO + np.arange(WI)[None, :]).ravel()
        xv = xv[:, idx]                                 # [C, WT*WI, d, h]
        xv = xv.reshape(NG, 2, CQ, WT, WI, DPC + 2, H + 2) \
               .transpose(0, 1, 2, 4, 3, 5, 6) \
               .reshape(NG, 2, KP, WT, DPC + 2, H + 2)
        xt = np.zeros((NG, 128, WT, DPC + 2, H + 2), xv.dtype)
        xt[:, 0: KP] = xv[:, 0]
        xt[:, 64: 64 + KP] = xv[:, 1]
        in_maps.append({
            "xt": np.ascontiguousarray(xt),
            "wt": wt, "pwk": pwk, "s1": s1, "b1": b1, "s2": s2, "b2": b2,
        })
    return in_maps


def _gather_output(results):
    z = np.empty((B, D, H, W, F), np.float32)
    for core in range(N_CORES):
        b = (core * DPC) // D
        d0 = (core * DPC) % D
        zc = results[core]["z"]                         # [F, NPOS]
        # free order was (w_o, t, d, h); w_global = t*WO + w_o
        zc = zc.reshape(F, WO, WT, DPC, H).transpose(3, 4, 2, 1, 0)
        z[b, d0: d0 + DPC] = zc.reshape(DPC, H, W, F)
    return z


def kernel(**inputs):
    global _COMPILED
    if _COMPILED is None:
        _COMPILED = _build_bass()
    in_maps = _prep_inputs(**inputs)
    res = run_bass_kernel_spmd(_COMPILED, in_maps,
                               core_ids=list(range(N_CORES)))
    return _gather_output(res.results)


if __name__ == "__main__":
    pass


# revision 25
# speedup vs baseline: 1.1288x; 1.1185x over previous
"""Trainium2 Bass kernel for DepthwiseSeparableConv3d (inference).

Problem: x[2,48,48,48,64] -> dw3x3x3 depthwise + BN + ReLU -> 1x1x1 conv
(64->128) + BN + ReLU -> z[2,48,48,48,128], all f32.

Strategy (8 NeuronCores, data-parallel over (b,d) slabs, 12 slabs/core):
 - Host pre-pads D (1-slab halo per side, zero at batch edges) and H/W
   (SAME padding) so the device kernel is a pure VALID 3x3x3 conv.
 - Depthwise conv runs on TensorE as a block-Toeplitz matmul:
   K=112 partitions = (8 channels x 14 W-inputs),
   M=96 partitions  = (8 channels x 12 W-outputs).
   The 3 W-taps live in the Toeplitz weight; the 9 (dz,dy) taps are
   PSUM-accumulated matmuls against free-dim-shifted views of the same
   SBUF tile (shifting free dims is free in an access pattern).
 - Weights are loaded once per tap-set via an explicit LDWEIGHTS; the
   matmuls are marked non-self-loading (ldweights=False) so the 8
   matmuls sharing a weight don't reload it.  Same for the pointwise
   weights (loaded exactly once).
 - BN1+ReLU is one ScalarE activation per (group, w-tile)
   (per-partition scale/bias).
 - A per-channel-group SBUF->SBUF DMA regroups (c,w)-partitions into
   pure-channel partitions (contiguous 576-elem blocks on both sides).
 - Pointwise 64->128 is a plain matmul; BN2+ReLU is one activation per
   4 position-chunks.
 - Output stays [f, positions] on device; host transposes to NDHWC.
"""

import os
import sys

for _p in ("/opt/trn_rl_repo", "/opt/pypackages"):
    if _p not in sys.path:
        sys.path.insert(0, _p)

import numpy as np
import ml_dtypes

import concourse.bass as bass
import concourse.tile as tile
from concourse import bacc, mybir
from concourse.bass_utils import run_bass_kernel_spmd

# ----- problem constants (hardcoded per spec) -----
B, D, H, W, C, F = 2, 48, 48, 48, 64, 128
EPS = 1e-3
N_CORES = 8
DPC = (B * D) // N_CORES          # d-slabs per core = 12
CG = 8                            # channels per depthwise group
NG = C // CG                      # 8 groups
WT = 4                            # W tiles
WO = W // WT                      # 12 outputs per W tile
WI = WO + 2                       # 14 inputs per W tile
KP = CG * WI                      # 112 K partitions
MP = CG * WO                      # 96 M partitions
DH = DPC * H                      # 576 (d,h) positions per W value
NHALF = 2                         # split (d,h) into two 288-col matmuls
NCOL = DH // NHALF                # 288
NPOS = DPC * H * W                # 27648 positions per core
ZCHUNK = 4                        # PW chunks per output DMA

BF16 = mybir.dt.bfloat16
F32 = mybir.dt.float32

_COMPILED = None


def _build_bass():
    nc = bacc.Bacc("TRN2", target_bir_lowering=False, debug=False,
                   num_devices=N_CORES)

    xt_d = nc.dram_tensor("xt", [NG, KP, WT, DPC + 2, H + 2], BF16,
                          kind="ExternalInput").ap()
    wt_d = nc.dram_tensor("wt", [NG, KP, 9, MP], BF16,
                          kind="ExternalInput").ap()
    pw_d = nc.dram_tensor("pwk", [C, F], BF16, kind="ExternalInput").ap()
    s1_d = nc.dram_tensor("s1", [MP, NG], F32, kind="ExternalInput").ap()
    b1_d = nc.dram_tensor("b1", [MP, NG], F32, kind="ExternalInput").ap()
    s2_d = nc.dram_tensor("s2", [F, 1], F32, kind="ExternalInput").ap()
    b2_d = nc.dram_tensor("b2", [F, 1], F32, kind="ExternalInput").ap()
    z_d = nc.dram_tensor("z", [F, NPOS], F32, kind="ExternalOutput").ap()

    with tile.TileContext(nc) as tc:
        with (
            tc.tile_pool(name="consts", bufs=1) as consts,
            tc.tile_pool(name="xt", bufs=NG) as xt_pool,
            tc.tile_pool(name="wt", bufs=NG) as wt_pool,
            tc.tile_pool(name="ybuf", bufs=3) as y_pool,
            tc.tile_pool(name="Ybig", bufs=1) as Y_pool,
            tc.tile_pool(name="zbuf", bufs=3) as z_pool,
        ):
            pw_sb = consts.tile([C, F], BF16)
            nc.sync.dma_start(pw_sb[:], pw_d[:])
            s1_sb = consts.tile([MP, NG], F32)
            nc.sync.dma_start(s1_sb[:], s1_d[:])
            b1_sb = consts.tile([MP, NG], F32)
            nc.sync.dma_start(b1_sb[:], b1_d[:])
            s2_sb = consts.tile([F, 1], F32)
            nc.sync.dma_start(s2_sb[:], s2_d[:])
            b2_sb = consts.tile([F, 1], F32)
            nc.sync.dma_start(b2_sb[:], b2_d[:])

            # Y: depthwise output in pure-channel layout.
            # free order (w_o, t, d, h): w_global = t*WO + w_o
            Y = Y_pool.tile([C, WO, WT, DPC, H], BF16)

            xg = []
            wg = []
            for g in range(NG):
                xg_t = xt_pool.tile([KP, WT, DPC + 2, H + 2], BF16, tag="xg")
                nc.sync.dma_start(xg_t[:], xt_d[g])
                wg_t = wt_pool.tile([KP, 9, MP], BF16, tag="wg")
                nc.sync.dma_start(wg_t[:], wt_d[g])
                xg.append(xg_t)
                wg.append(wg_t)

            with tc.tile_pool(name="psdw", bufs=4, space="PSUM") as ps_pool:
                for g in range(NG):
                    yg = y_pool.tile([MP, WT, DPC, H], BF16, tag="yg")
                    # one PSUM tile per t; 2 nh slices bank-aligned at 512
                    ps = [ps_pool.tile([MP, NHALF, 512], F32, tag="ps",
                                       name=f"ps_{t}") for t in range(WT)]
                    for izy, (dz, dy) in enumerate(
                            (a, b) for a in range(3) for b in range(3)):
                        nc.tensor.ldweights(wg[g][:, izy, :])
                        for t in range(WT):
                            for nh in range(NHALF):
                                d0 = nh * (DPC // NHALF)
                                rhs = xg[g][:, t,
                                            dz + d0: dz + d0 + DPC // NHALF,
                                            dy: dy + H]
                                mm = nc.tensor.matmul(
                                    ps[t][:, nh, 0:NCOL],
                                    wg[g][:, izy, :],
                                    rhs,
                                    start=(izy == 0),
                                    stop=(izy == 8),
                                )
                                mm.ins.ldweights = False
                    for t in range(WT):
                        out_v = yg[:, t].rearrange("c d h -> c (d h)") \
                                        .rearrange("c (n r) -> c n r",
                                                   n=NHALF, r=NCOL)
                        nc.scalar.activation(
                            out_v,
                            ps[t][:, :, 0:NCOL],
                            mybir.ActivationFunctionType.Relu,
                            bias=b1_sb[:, g: g + 1],
                            scale=s1_sb[:, g: g + 1],
                        )
                    # regroup (c,w)-partitions -> channel partitions.
                    # src iter: (c, w_o, t, d, h) == dst free layout order
                    nc.sync.dma_start(Y[g * CG:(g + 1) * CG], yg[:])

            # pointwise + BN2 + ReLU, chunked over positions
            n_chunks = NPOS // NCOL          # 96
            Yf = Y.rearrange("c w t d h -> c (w t d h)")
            zf = z_d.rearrange("f (j n) -> f j n", j=n_chunks, n=NCOL)
            with tc.tile_pool(name="pspw", bufs=2, space="PSUM") as pw_pool:
                first_pw = True
                for j0 in range(0, n_chunks, ZCHUNK):
                    pps = pw_pool.tile([F, ZCHUNK, 512], F32, tag="pwps")
                    zt = z_pool.tile([F, ZCHUNK, NCOL], F32, tag="zt")
                    for jj in range(ZCHUNK):
                        j = j0 + jj
                        if first_pw:
                            nc.tensor.ldweights(pw_sb[:])
                            first_pw = False
                        mm = nc.tensor.matmul(
                            pps[:, jj, 0:NCOL], pw_sb[:],
                            Yf[:, j * NCOL:(j + 1) * NCOL],
                            start=True, stop=True)
                        mm.ins.ldweights = False
                    nc.scalar.activation(
                        zt[:], pps[:, :, 0:NCOL],
                        mybir.ActivationFunctionType.Relu,
                        bias=b2_sb[:, 0:1], scale=s2_sb[:, 0:1])
                    nc.sync.dma_start(zf[:, j0: j0 + ZCHUNK, :], zt[:])

    nc.compile()
    return nc


def _prep_inputs(x, dw_kernel, dw_bias, bn1_gamma, bn1_beta, bn1_mean,
                 bn1_var, pw_kernel, pw_bias, bn2_gamma, bn2_beta, bn2_mean,
                 bn2_var):
    """Build per-core input maps (numpy only, off the device clock)."""
    x = np.asarray(x, np.float32)
    dw_kernel = np.asarray(dw_kernel, np.float32)
    dw_bias = np.asarray(dw_bias, np.float32)
    bn1_gamma = np.asarray(bn1_gamma, np.float32)
    bn1_beta = np.asarray(bn1_beta, np.float32)
    bn1_mean = np.asarray(bn1_mean, np.float32)
    bn1_var = np.asarray(bn1_var, np.float32)
    pw_kernel = np.asarray(pw_kernel, np.float32)
    pw_bias = np.asarray(pw_bias, np.float32)
    bn2_gamma = np.asarray(bn2_gamma, np.float32)
    bn2_beta = np.asarray(bn2_beta, np.float32)
    bn2_mean = np.asarray(bn2_mean, np.float32)
    bn2_var = np.asarray(bn2_var, np.float32)
    a1 = bn1_gamma / np.sqrt(bn1_var + EPS)
    c1 = a1 * (dw_bias - bn1_mean) + bn1_beta
    a2 = bn2_gamma / np.sqrt(bn2_var + EPS)
    c2 = a2 * (pw_bias - bn2_mean) + bn2_beta

    # depthwise Toeplitz weights: [NG, KP, 9, MP]
    dw = dw_kernel[:, :, :, 0, :]                       # [3,3,3,C]
    wt = np.zeros((NG, KP, 9, MP), np.float32)
    for ci in range(CG):
        for wo in range(WO):
            for dx in range(3):
                # wt[g, ci*WI + wo+dx, (dz*3+dy), ci*WO + wo] = dw[dz,dy,dx,c]
                wt[:, ci * WI + wo + dx, :, ci * WO + wo] = (
                    dw[:, :, dx, :].reshape(9, NG, CG)[:, :, ci].T)
    wt = wt.astype(ml_dtypes.bfloat16)

    # scale/bias vectors in (c-major, w) partition order: m = ci*WO + wo
    s1 = np.zeros((MP, NG), np.float32)
    b1 = np.zeros((MP, NG), np.float32)
    for g in range(NG):
        for ci in range(CG):
            s1[ci * WO:(ci + 1) * WO, g] = a1[g * CG + ci]
            b1[ci * WO:(ci + 1) * WO, g] = c1[g * CG + ci]

    pwk = pw_kernel.astype(ml_dtypes.bfloat16)
    s2 = a2.reshape(F, 1).astype(np.float32)
    b2 = c2.reshape(F, 1).astype(np.float32)

    # x padded once globally: [B, D+2, H+2, W+2, C]
    xp = np.zeros((B, D + 2, H + 2, W + 2, C), np.float32)
    xp[:, 1:-1, 1:-1, 1:-1, :] = x
    xp = xp.astype(ml_dtypes.bfloat16)

    in_maps = []
    for core in range(N_CORES):
        b = (core * DPC) // D
        d0 = (core * DPC) % D
        sl = xp[b, d0: d0 + DPC + 2]                    # [14, 50, 50, C]
        # xt[g, ci*WI+wi, t, d, h] = sl[d, h, 12t+wi, 8g+ci]
        xv = np.ascontiguousarray(sl.transpose(3, 2, 0, 1))  # [C, w50, d, h]
        # build overlapping w-tiles: index w = t*WO + wi
        idx = (np.arange(WT)[:, None] * WO + np.arange(WI)[None, :]).ravel()
        xv = xv[:, idx]                                 # [C, WT*WI, d, h]
        xt = xv.reshape(NG, CG, WT, WI, DPC + 2, H + 2) \
               .transpose(0, 1, 3, 2, 4, 5) \
               .reshape(NG, KP, WT, DPC + 2, H + 2)
        in_maps.append({
            "xt": np.ascontiguousarray(xt),
            "wt": wt, "pwk": pwk, "s1": s1, "b1": b1, "s2": s2, "b2": b2,
        })
    return in_maps


def _gather_output(results):
    z = np.empty((B, D, H, W, F), np.float32)
    for core in range(N_CORES):
        b = (core * DPC) // D
        d0 = (core * DPC) % D
        zc = results[core]["z"]                         # [F, NPOS]
        # free order was (w_o, t, d, h); w_global = t*WO + w_o
        zc = zc.reshape(F, WO, WT, DPC, H).transpose(3, 4, 2, 1, 0)
        z[b, d0: d0 + DPC] = zc.reshape(DPC, H, W, F)
    return z


def kernel(**inputs):
    global _COMPILED
    if _COMPILED is None:
        _COMPILED = _build_bass()
    in_maps = _prep_inputs(**inputs)
    res = run_bass_kernel_spmd(_COMPILED, in_maps,
                               core_ids=list(range(N_CORES)))
    return _gather_output(res.results)


if __name__ == "__main__":
    pass


# revision 28
# speedup vs baseline: 1.1636x; 1.0308x over previous
"""Trainium2 Bass kernel for DepthwiseSeparableConv3d (inference).

Problem: x[2,48,48,48,64] -> dw3x3x3 depthwise + BN + ReLU -> 1x1x1 conv
(64->128) + BN + ReLU -> z[2,48,48,48,128], all f32.

Strategy (8 NeuronCores, data-parallel over (b,d) slabs, 12 slabs/core):
 - Host pre-pads D (1-slab halo per side, zero at batch edges) and H/W
   (SAME padding) so the device kernel is a pure VALID 3x3x3 conv.
 - Depthwise conv runs on TensorE as a block-Toeplitz matmul:
   K=112 partitions = (8 channels x 14 W-inputs),
   M=96 partitions  = (8 channels x 12 W-outputs).
   The 3 W-taps live in the Toeplitz weight; the 9 (dz,dy) taps are
   PSUM-accumulated matmuls against free-dim-shifted views of the same
   SBUF tile (shifting free dims is free in an access pattern).
 - Weights are loaded once per tap-set via an explicit LDWEIGHTS; the
   matmuls are marked non-self-loading (ldweights=False) so the 8
   matmuls sharing a weight don't reload it.  Same for the pointwise
   weights (loaded exactly once).
 - BN1+ReLU is one ScalarE activation per (group, w-tile)
   (per-partition scale/bias).
 - A per-channel-group SBUF->SBUF DMA regroups (c,w)-partitions into
   pure-channel partitions (contiguous 576-elem blocks on both sides).
 - Pointwise 64->128 is a plain matmul; BN2+ReLU is one activation per
   4 position-chunks.
 - Output stays [f, positions] on device; host transposes to NDHWC.
"""

import os
import sys

for _p in ("/opt/trn_rl_repo", "/opt/pypackages"):
    if _p not in sys.path:
        sys.path.insert(0, _p)

import numpy as np
import ml_dtypes

import concourse.bass as bass
import concourse.tile as tile
from concourse import bacc, mybir
from concourse.bass_utils import run_bass_kernel_spmd

# ----- problem constants (hardcoded per spec) -----
B, D, H, W, C, F = 2, 48, 48, 48, 64, 128
EPS = 1e-3
N_CORES = 8
DPC = (B * D) // N_CORES          # d-slabs per core = 12
CG = 8                            # channels per depthwise group
NG = C // CG                      # 8 groups
WT = 4                            # W tiles
WO = W // WT                      # 12 outputs per W tile
WI = WO + 2                       # 14 inputs per W tile
KP = CG * WI                      # 112 K partitions
MP = CG * WO                      # 96 M partitions
DH = DPC * H                      # 576 (d,h) positions per W value
NHALF = 2                         # split (d,h) into two 288-col matmuls
NCOL = DH // NHALF                # 288
NPOS = DPC * H * W                # 27648 positions per core
ZCHUNK = 4                        # PW chunks per output DMA

BF16 = mybir.dt.bfloat16
F32 = mybir.dt.float32

_COMPILED = None


def _build_bass():
    nc = bacc.Bacc("TRN2", target_bir_lowering=False, debug=False,
                   num_devices=N_CORES)

    xt_d = nc.dram_tensor("xt", [NG, KP, WT, DPC + 2, H + 2], BF16,
                          kind="ExternalInput").ap()
    wt_d = nc.dram_tensor("wt", [NG, KP, 9, MP], BF16,
                          kind="ExternalInput").ap()
    pw_d = nc.dram_tensor("pwk", [C, F], BF16, kind="ExternalInput").ap()
    s1_d = nc.dram_tensor("s1", [MP, NG], F32, kind="ExternalInput").ap()
    b1_d = nc.dram_tensor("b1", [MP, NG], F32, kind="ExternalInput").ap()
    s2_d = nc.dram_tensor("s2", [F, 1], F32, kind="ExternalInput").ap()
    b2_d = nc.dram_tensor("b2", [F, 1], F32, kind="ExternalInput").ap()
    z_d = nc.dram_tensor("z", [F, NPOS], F32, kind="ExternalOutput").ap()

    with tile.TileContext(nc) as tc:
        with (
            tc.tile_pool(name="consts", bufs=1) as consts,
            tc.tile_pool(name="xt", bufs=NG) as xt_pool,
            tc.tile_pool(name="wt", bufs=NG) as wt_pool,
            tc.tile_pool(name="ybuf", bufs=3) as y_pool,
            tc.tile_pool(name="Ybig", bufs=1) as Y_pool,
            tc.tile_pool(name="zbuf", bufs=3) as z_pool,
        ):
            pw_sb = consts.tile([C, F], BF16)
            nc.sync.dma_start(pw_sb[:], pw_d[:])
            s1_sb = consts.tile([MP, NG], F32)
            nc.sync.dma_start(s1_sb[:], s1_d[:])
            b1_sb = consts.tile([MP, NG], F32)
            nc.sync.dma_start(b1_sb[:], b1_d[:])
            s2_sb = consts.tile([F, 1], F32)
            nc.sync.dma_start(s2_sb[:], s2_d[:])
            b2_sb = consts.tile([F, 1], F32)
            nc.sync.dma_start(b2_sb[:], b2_d[:])

            # Y: depthwise output in pure-channel layout, t-major so the
            # per-tile pointwise reads contiguous 6912-column spans.
            Y = Y_pool.tile([C, WT, WO, DPC, H], BF16)

            xg = []
            wg = []
            for g in range(NG):
                xg_t = xt_pool.tile([KP, WT, DPC + 2, H + 2], BF16, tag="xg")
                nc.sync.dma_start(xg_t[:], xt_d[g])
                wg_t = wt_pool.tile([KP, 9, MP], BF16, tag="wg")
                nc.sync.dma_start(wg_t[:], wt_d[g])
                xg.append(xg_t)
                wg.append(wg_t)

            # per-tile flat position count and PW chunking
            TPOS = WO * DPC * H              # 6912
            NPW = 432                        # 16 uniform PW chunks per tile
            NQ = TPOS // NPW                 # 16
            zf = z_d.rearrange("f (t q n) -> f t q n", t=WT, q=NQ, n=NPW)

            with (
                tc.tile_pool(name="psdw", bufs=2, space="PSUM") as ps_pool,
                tc.tile_pool(name="pspw", bufs=2, space="PSUM") as pw_pool,
            ):
                for t in range(WT):
                    for g in range(NG):
                        yg = y_pool.tile([MP, DPC, H], BF16, tag="yg")
                        ps = ps_pool.tile([MP, NHALF, 512], F32, tag="ps")
                        for izy, (dz, dy) in enumerate(
                                (a, b) for a in range(3) for b in range(3)):
                            for nh in range(NHALF):
                                d0 = nh * (DPC // NHALF)
                                rhs = xg[g][:, t,
                                            dz + d0: dz + d0 + DPC // NHALF,
                                            dy: dy + H]
                                nc.tensor.matmul(
                                    ps[:, nh, 0:NCOL],
                                    wg[g][:, izy, :],
                                    rhs,
                                    start=(izy == 0),
                                    stop=(izy == 8),
                                )
                        out_v = yg[:].rearrange("c d h -> c (d h)") \
                                     .rearrange("c (n r) -> c n r",
                                                n=NHALF, r=NCOL)
                        nc.scalar.activation(
                            out_v,
                            ps[:, :, 0:NCOL],
                            mybir.ActivationFunctionType.Relu,
                            bias=b1_sb[:, g: g + 1],
                            scale=s1_sb[:, g: g + 1],
                        )
                        # regroup (c,w)-partitions -> channel partitions
                        nc.sync.dma_start(Y[g * CG:(g + 1) * CG, t], yg[:])

                    # pointwise for this tile (overlaps next tile's DW)
                    Yt = Y[:, t].rearrange("c w d h -> c (w d h)")
                    for q0 in range(0, NQ, 2):
                        pps = pw_pool.tile([F, 2, 512], F32, tag="pwps")
                        zt = z_pool.tile([F, 2, NPW], F32, tag="zt")
                        for qq in range(2):
                            q = q0 + qq
                            nc.tensor.matmul(
                                pps[:, qq, 0:NPW], pw_sb[:],
                                Yt[:, q * NPW:(q + 1) * NPW],
                                start=True, stop=True)
                        nc.scalar.activation(
                            zt[:], pps[:, :, 0:NPW],
                            mybir.ActivationFunctionType.Relu,
                            bias=b2_sb[:, 0:1], scale=s2_sb[:, 0:1])
                        nc.sync.dma_start(zf[:, t, q0: q0 + 2, :], zt[:])

    nc.compile()
    return nc


def _prep_inputs(x, dw_kernel, dw_bias, bn1_gamma, bn1_beta, bn1_mean,
                 bn1_var, pw_kernel, pw_bias, bn2_gamma, bn2_beta, bn2_mean,
                 bn2_var):
    """Build per-core input maps (numpy only, off the device clock)."""
    x = np.asarray(x, np.float32)
    dw_kernel = np.asarray(dw_kernel, np.float32)
    dw_bias = np.asarray(dw_bias, np.float32)
    bn1_gamma = np.asarray(bn1_gamma, np.float32)
    bn1_beta = np.asarray(bn1_beta, np.float32)
    bn1_mean = np.asarray(bn1_mean, np.float32)
    bn1_var = np.asarray(bn1_var, np.float32)
    pw_kernel = np.asarray(pw_kernel, np.float32)
    pw_bias = np.asarray(pw_bias, np.float32)
    bn2_gamma = np.asarray(bn2_gamma, np.float32)
    bn2_beta = np.asarray(bn2_beta, np.float32)
    bn2_mean = np.asarray(bn2_mean, np.float32)
    bn2_var = np.asarray(bn2_var, np.float32)
    a1 = bn1_gamma / np.sqrt(bn1_var + EPS)
    c1 = a1 * (dw_bias - bn1_mean) + bn1_beta
    a2 = bn2_gamma / np.sqrt(bn2_var + EPS)
    c2 = a2 * (pw_bias - bn2_mean) + bn2_beta

    # depthwise Toeplitz weights: [NG, KP, 9, MP]
    dw = dw_kernel[:, :, :, 0, :]                       # [3,3,3,C]
    wt = np.zeros((NG, KP, 9, MP), np.float32)
    for ci in range(CG):
        for wo in range(WO):
            for dx in range(3):
                # wt[g, ci*WI + wo+dx, (dz*3+dy), ci*WO + wo] = dw[dz,dy,dx,c]
                wt[:, ci * WI + wo + dx, :, ci * WO + wo] = (
                    dw[:, :, dx, :].reshape(9, NG, CG)[:, :, ci].T)
    wt = wt.astype(ml_dtypes.bfloat16)

    # scale/bias vectors in (c-major, w) partition order: m = ci*WO + wo
    s1 = np.zeros((MP, NG), np.float32)
    b1 = np.zeros((MP, NG), np.float32)
    for g in range(NG):
        for ci in range(CG):
            s1[ci * WO:(ci + 1) * WO, g] = a1[g * CG + ci]
            b1[ci * WO:(ci + 1) * WO, g] = c1[g * CG + ci]

    pwk = pw_kernel.astype(ml_dtypes.bfloat16)
    s2 = a2.reshape(F, 1).astype(np.float32)
    b2 = c2.reshape(F, 1).astype(np.float32)

    # x padded once globally: [B, D+2, H+2, W+2, C]
    xp = np.zeros((B, D + 2, H + 2, W + 2, C), np.float32)
    xp[:, 1:-1, 1:-1, 1:-1, :] = x
    xp = xp.astype(ml_dtypes.bfloat16)

    in_maps = []
    for core in range(N_CORES):
        b = (core * DPC) // D
        d0 = (core * DPC) % D
        sl = xp[b, d0: d0 + DPC + 2]                    # [14, 50, 50, C]
        # xt[g, ci*WI+wi, t, d, h] = sl[d, h, 12t+wi, 8g+ci]
        xv = np.ascontiguousarray(sl.transpose(3, 2, 0, 1))  # [C, w50, d, h]
        # build overlapping w-tiles: index w = t*WO + wi
        idx = (np.arange(WT)[:, None] * WO + np.arange(WI)[None, :]).ravel()
        xv = xv[:, idx]                                 # [C, WT*WI, d, h]
        xt = xv.reshape(NG, CG, WT, WI, DPC + 2, H + 2) \
               .transpose(0, 1, 3, 2, 4, 5) \
               .reshape(NG, KP, WT, DPC + 2, H + 2)
        in_maps.append({
            "xt": np.ascontiguousarray(xt),
            "wt": wt, "pwk": pwk, "s1": s1, "b1": b1, "s2": s2, "b2": b2,
        })
    return in_maps


def _gather_output(results):
    z = np.empty((B, D, H, W, F), np.float32)
    for core in range(N_CORES):
        b = (core * DPC) // D
        d0 = (core * DPC) % D
        zc = results[core]["z"]                         # [F, NPOS]
        # free order was (t, w_o, d, h); w_global = t*WO + w_o
        zc = zc.reshape(F, WT, WO, DPC, H).transpose(3, 4, 1, 2, 0)
        z[b, d0: d0 + DPC] = zc.reshape(DPC, H, W, F)
    return z


def kernel(**inputs):
    global _COMPILED
    if _COMPILED is None:
        _COMPILED = _build_bass()
    in_maps = _prep_inputs(**inputs)
    res = run_bass_kernel_spmd(_COMPILED, in_maps,
                               core_ids=list(range(N_CORES)))
    return _gather_output(res.results)


if __name__ == "__main__":
    pass
